# revision 22
# baseline (speedup 1.0000x reference)
"""Multi-head attention (B=2, N=2048, D=1024, H=16) on 8 TRN2 NeuronCores.

Sharding: tensor-parallel over heads across all 8 cores. Core i handles
heads [2i, 2i+2) (128 of the 1024 QKV output dims) for BOTH batches.
After local attention, one 8-core AllToAll re-shards from head-split to
(batch, sequence-slab)-split, so core j computes the output projection
for batch j//4, rows [512*(j%4), 512*(j%4+1)). Host-side work is pure
concatenation.

Layout notes:
  - x[b] is transposed once on-chip (PE transpose) to xT [D, N]; Q^T, K^T
    and V^T are then produced directly by matmuls against xT, and V^T is
    PE-transposed back to V's [k, d] layout for the AV matmul.
  - Scores are computed transposed (S^T [k, q]) so exp(S^T) tiles feed the
    AV matmul as the moving operand with k on partitions.
  - V gets an appended ones column per head, so the AV matmul also emits
    softmax denominators (row 64 of the [65, q] accumulator) for free.
  - All big matmuls run in bf16 (inputs rounded on write by the producing
    engine; fp32 PSUM accumulation). Verified ~2e-3 end-to-end rel err.
"""

import numpy as np

import concourse.bass as bass
import concourse.mybir as mybir
import concourse.tile as tile
from concourse import bacc
from concourse.bass_utils import run_bass_kernel_spmd
from concourse.masks import make_identity

F32 = mybir.dt.float32
BF16 = mybir.dt.bfloat16
EXP = mybir.ActivationFunctionType.Exp
IDENT = mybir.ActivationFunctionType.Identity
BYPASS = mybir.AluOpType.bypass

P = 128
B, N, D = 2, 2048, 1024
NH, HD = 16, 64
TP = 8                  # head-parallel group size (all cores)
HL = D // TP            # 128 local head dims (2 heads x 64)
NHL = NH // TP          # 2 local heads
QS = 512                # query slab width
NQS = N // QS           # 4 slabs
KC = N // P             # 16 key chunks of 128
DC = D // P             # 8 model-dim chunks of 128
ROWS = 512              # output rows per core (one slab of one batch)
RG = [[0, 1, 2, 3, 4, 5, 6, 7]]
SCALE = 1.0 / np.sqrt(HD)

_CACHE = {}


def build_nc():
    nc = bacc.Bacc("TRN2", target_bir_lowering=False, debug=False,
                   num_devices=8)

    x_ext = nc.declare_dram_parameter("x", [B, N, D], F32, isOutput=False)
    wq_ext = nc.declare_dram_parameter("wq", [D, HL], F32, isOutput=False)
    bq_ext = nc.declare_dram_parameter("bq", [HL], F32, isOutput=False)
    wk_ext = nc.declare_dram_parameter("wk", [D, HL], F32, isOutput=False)
    bk_ext = nc.declare_dram_parameter("bk", [HL], F32, isOutput=False)
    wv_ext = nc.declare_dram_parameter("wv", [D, HL], F32, isOutput=False)
    bv_ext = nc.declare_dram_parameter("bv", [HL], F32, isOutput=False)
    wo_ext = nc.declare_dram_parameter("wo", [D, D], F32, isOutput=False)
    bo_ext = nc.declare_dram_parameter("bo", [D], F32, isOutput=False)
    out_ext = nc.declare_dram_parameter("out", [ROWS, D], F32, isOutput=True)

    with tile.TileContext(nc) as tc:
        with (
            tc.tile_pool(name="const", bufs=1) as const,
            tc.tile_pool(name="persist", bufs=1) as persist,
            tc.tile_pool(name="dram", bufs=1, space="DRAM") as dram,
        ):
            identity = const.tile([P, P], F32)
            make_identity(nc, identity)
            ones128 = const.tile([1, P], F32)
            nc.gpsimd.memset(ones128, 1.0)
            ones64b = const.tile([1, HD], BF16)
            nc.gpsimd.memset(ones64b, 1.0)
            identity_b = const.tile([P, P], BF16)
            make_identity(nc, identity_b)

            # persistent SBUF tensors
            QT = persist.tile([P, B, N], BF16)        # [128 d, b, 2048 q]
            KT = persist.tile([P, B, N], BF16)        # [128 d, b, 2048 k]
            Vaug = persist.tile([P, B, KC, NHL, HD + 1], BF16)
            nc.gpsimd.memset(Vaug[:, :, :, :, HD:HD + 1], 1.0)

            a2a_in = dram.tile([TP, HL, QS], BF16)
            a2a_out = dram.tile([TP, HL, QS], BF16)

            # ---------------- phase 1: xT + QKV projections ----------------
            with (
                tc.tile_pool(name="xtp", bufs=1) as xtp,
                tc.tile_pool(name="wp", bufs=1) as wp,
                tc.tile_pool(name="tmp1", bufs=3) as tmp1,
                tc.tile_pool(name="vtp", bufs=2) as vtp,
                tc.tile_pool(name="ps1", bufs=2, space="PSUM") as ps1,
                tc.tile_pool(name="ps1t", bufs=3, space="PSUM") as ps1t,
            ):
                wq_sb = wp.tile([P, DC, HL], BF16)
                wk_sb = wp.tile([P, DC, HL], BF16)
                wv_sb = wp.tile([P, DC, HL], BF16)
                for w_sb, w_ext in ((wq_sb, wq_ext), (wk_sb, wk_ext),
                                    (wv_sb, wv_ext)):
                    w_raw = tmp1.tile([P, DC, HL], F32, tag="wraw")
                    nc.sync.dma_start(w_raw, w_ext[:].rearrange("(c p) n -> p c n", p=P))
                    nc.vector.tensor_copy(w_sb, w_raw)

                bqs = wp.tile([P, 1], F32)
                bks = wp.tile([P, 1], F32)
                nc.sync.dma_start(bqs, bq_ext[:].rearrange("(c p) -> p c", p=P))
                nc.sync.dma_start(bks, bk_ext[:].rearrange("(c p) -> p c", p=P))
                # scores use (q*s + bq*s) . k + bk form: scale Q side only
                nc.vector.tensor_scalar_mul(bqs, bqs, SCALE)

                bv_sb = wp.tile([1, HL], F32)
                nc.sync.dma_start(bv_sb, bv_ext[:].rearrange("(o n) -> o n", o=1))
                bv_ps = ps1.tile([P, HL], F32, tag="psm")
                nc.tensor.matmul(bv_ps, lhsT=ones128, rhs=bv_sb,
                                 start=True, stop=True)
                bv_bc = wp.tile([P, HL], F32)
                nc.vector.tensor_copy(bv_bc, bv_ps)

                for b in range(B):
                    xT = xtp.tile([P, DC, N], BF16)   # [1024 D, 2048 q]
                    # transpose x[b] -> xT
                    for qc in range(N // P):
                        x_t = tmp1.tile([P, D], F32)
                        nc.sync.dma_start(x_t, x_ext[b, qc * P:(qc + 1) * P, :])
                        for dc in range(DC):
                            pst = ps1t.tile([P, P], F32, tag="pst")
                            nc.tensor.transpose(
                                pst, x_t[:, dc * P:(dc + 1) * P], identity)
                            nc.vector.tensor_copy(
                                xT[:, dc, qc * P:(qc + 1) * P], pst)

                    # Q^T, K^T : [128 d, 2048], d on partitions
                    for w_sb, bias, scl, dst in (
                        (wq_sb, bqs, SCALE, QT),
                        (wk_sb, bks, 1.0, KT),
                    ):
                        for qs in range(NQS):
                            psm = ps1.tile([P, QS], F32)
                            for dc in range(DC):
                                nc.tensor.matmul(
                                    psm,
                                    lhsT=w_sb[:, dc, :],
                                    rhs=xT[:, dc, qs * QS:(qs + 1) * QS],
                                    start=(dc == 0), stop=(dc == DC - 1))
                            nc.scalar.activation(
                                dst[:, b, qs * QS:(qs + 1) * QS], psm,
                                IDENT, bias=bias[:, 0:1], scale=scl)

                    # V^T : [128 d, 2048 k], then PE-transpose to [k, d]
                    for ks in range(NQS):
                        psm = ps1.tile([P, QS], F32)
                        for dc in range(DC):
                            nc.tensor.matmul(
                                psm,
                                lhsT=wv_sb[:, dc, :],
                                rhs=xT[:, dc, ks * QS:(ks + 1) * QS],
                                start=(dc == 0), stop=(dc == DC - 1))
                        vt_t = vtp.tile([P, QS], BF16)
                        nc.scalar.copy(vt_t, psm)
                        for kk in range(QS // P):
                            kc = ks * (QS // P) + kk
                            pst = ps1t.tile([P, P], BF16, tag="pst")
                            nc.tensor.transpose(
                                pst, vt_t[:, kk * P:(kk + 1) * P], identity_b)
                            nc.vector.tensor_add(
                                out=Vaug[:, b, kc, :, :HD],
                                in0=pst[:].rearrange("p (h d) -> p h d", d=HD),
                                in1=bv_bc[:].rearrange("p (h d) -> p h d", d=HD))

            # ---------------- phase 2: attention ----------------
            with (
                tc.tile_pool(name="wo_p", bufs=1) as wo_p,
                tc.tile_pool(name="ptp", bufs=4) as ptp,
                tc.tile_pool(name="nrm", bufs=3) as nrm,
                tc.tile_pool(name="ps_s", bufs=3, space="PSUM") as ps_s,
                tc.tile_pool(name="ps_o", bufs=2, space="PSUM") as ps_o,
                tc.tile_pool(name="ps_b", bufs=2, space="PSUM") as ps_b,
            ):
                # prefetch wo + bo while attention runs
                wo_sb = wo_p.tile([P, DC, D], BF16)
                wo_raw = wo_p.tile([P, DC, D], F32)
                nc.sync.dma_start(wo_raw, wo_ext[:].rearrange("(c p) n -> p c n", p=P))
                nc.vector.tensor_copy(wo_sb, wo_raw)
                bo_sb = wo_p.tile([1, D], F32)
                nc.sync.dma_start(bo_sb, bo_ext[:].rearrange("(o n) -> o n", o=1))
                bo_bc = wo_p.tile([P, D], F32)
                for oc in range(2):
                    bo_ps = ps_b.tile([P, QS], F32, tag="bcp")
                    nc.tensor.matmul(bo_ps, lhsT=ones128,
                                     rhs=bo_sb[:, oc * QS:(oc + 1) * QS],
                                     start=True, stop=True)
                    nc.vector.tensor_copy(bo_bc[:, oc * QS:(oc + 1) * QS], bo_ps)

                for b in range(B):
                    for h in range(NHL):
                        po = h * HD
                        for qs in range(NQS):
                            j = b * NQS + qs      # a2a destination core
                            acc = ps_o.tile([P, QS], F32, tag="acc")
                            for kc in range(KC):
                                pss = ps_s.tile([P, QS], F32, tag="pss")
                                nc.tensor.matmul(
                                    pss,
                                    lhsT=KT[po:po + HD, b, kc * P:(kc + 1) * P],
                                    rhs=QT[po:po + HD, b, qs * QS:(qs + 1) * QS],
                                    start=True, stop=True)
                                pt = ptp.tile([P, QS], BF16)
                                nc.scalar.activation(pt, pss, EXP)
                                nc.tensor.matmul(
                                    acc[:HD + 1],
                                    lhsT=Vaug[:, b, kc, h, :],
                                    rhs=pt,
                                    start=(kc == 0), stop=(kc == KC - 1))
                            rec = nrm.tile([1, QS], BF16)
                            with nc.allow_low_precision(
                                    reason="softmax denom reciprocal to bf16"):
                                nc.vector.reciprocal(rec, acc[HD:HD + 1])
                            bcp = ps_b.tile([P, QS], F32, tag="bcp")
                            nc.tensor.matmul(bcp[:HD], lhsT=ones64b,
                                             rhs=rec, start=True, stop=True)
                            bc_sb = nrm.tile([HD, QS], F32, tag="bcsb")
                            nc.scalar.copy(bc_sb, bcp[:HD])
                            onrm = nrm.tile([HD, QS], BF16, tag="onrm")
                            nc.vector.tensor_mul(onrm, acc[:HD], bc_sb)
                            nc.sync.dma_start(
                                a2a_in[j, po:po + HD, :], onrm)

                # ---------------- phase 3: AllToAll ----------------
                nc.gpsimd.collective_compute(
                    "AllToAll", BYPASS,
                    ins=[a2a_in[:].opt()],
                    outs=[a2a_out[:].opt()],
                    replica_groups=RG)

                # ---------------- phase 4: output projection ----------------
                ot_sb = wo_p.tile([P, DC, QS], BF16)
                nc.sync.dma_start(ot_sb, a2a_out[:].rearrange("s p q -> p s q"))
                for mq in range(ROWS // P):
                    for oc in range(2):
                        psm = ps_s.tile([P, QS], F32, tag="pss")
                        for dc in range(DC):
                            nc.tensor.matmul(
                                psm,
                                lhsT=ot_sb[:, dc, mq * P:(mq + 1) * P],
                                rhs=wo_sb[:, dc, oc * QS:(oc + 1) * QS],
                                start=(dc == 0), stop=(dc == DC - 1))
                        o_t = nrm.tile([P, QS], F32)
                        nc.vector.tensor_add(
                            out=o_t, in0=psm,
                            in1=bo_bc[:, oc * QS:(oc + 1) * QS])
                        nc.sync.dma_start(
                            out_ext[mq * P:(mq + 1) * P,
                                    oc * QS:(oc + 1) * QS], o_t)

    nc.finalize()
    return nc


def make_in_maps(inputs):
    xx = np.ascontiguousarray(np.asarray(inputs["x"], dtype=np.float32))
    full = {k: np.asarray(inputs[k], dtype=np.float32)
            for k in ("wq", "bq", "wk", "bk", "wv", "bv", "wo", "bo")}
    in_maps = []
    for i in range(8):
        hs = i * HL
        m = {"x": xx}
        for k in ("wq", "wk", "wv"):
            m[k] = np.ascontiguousarray(full[k][:, hs:hs + HL])
        for k in ("bq", "bk", "bv"):
            m[k] = np.ascontiguousarray(full[k][hs:hs + HL])
        m["wo"] = np.ascontiguousarray(full["wo"])
        m["bo"] = np.ascontiguousarray(full["bo"])
        in_maps.append(m)
    return in_maps


def kernel(**inputs):
    if "nc" not in _CACHE:
        _CACHE["nc"] = build_nc()
    nc = _CACHE["nc"]
    in_maps = make_in_maps(inputs)
    res = run_bass_kernel_spmd(nc, in_maps, core_ids=list(range(8)))
    out = np.empty((B, N, D), dtype=np.float32)
    for j in range(8):
        b, t = j // NQS, j % NQS
        out[b, t * ROWS:(t + 1) * ROWS] = res.results[j]["out"]
    return out
